# revision 19
# baseline (speedup 1.0000x reference)
"""GAT attention layer (gnn_message_passing) on 8 TRN2 NeuronCores.

Math (reference):
    h = inp @ W;  s1 = h @ a1;  s2 = h @ a2
    e = leaky_relu(s1 + s2^T, 0.2);  scores = where(adj>0, e, -9e15)
    out = elu(softmax_row(scores) @ h)

Device algorithm (per core, rows R = N/8):
  Softmax without max-subtraction; masked entries contribute exactly 0 via
  P = adj * exp(e):
      out[i,:] = elu( (sum_j P[i,j] h[j,:]) / (sum_j P[i,j]) )
  Key identity:  exp(lrelu(x)) = exp(0.2*s1) * exp(0.2*s2) * exp(0.8*relu(x))
  with x = s1[i]+s2[j].  The exp(0.2*s1[i]) factor is constant along the
  softmax row, so it cancels between numerator and denominator and is
  simply dropped.  The 0.2*s2[j] term is per-partition and rides the
  activation bias slot.  Per element this leaves:
      x+ = max(s1[i] + s2[j], 0)        [DVE tensor_scalar or ACT Relu]
      e  = Exp(0.8*x+ + 0.2*s2[j])      [ACT, bf16 out]
      pT = e * adjT                     [DVE tensor_tensor, bf16]
  Everything is in the TRANSPOSED orientation [j (partitions), i (free)]
  so the attention matmul needs no on-chip transposes.  num|denom in one
  bf16 matmul: rhs = [h | ones] (257 cols), lhsT = P^T slices.
  s2 and 0.2*s2 come out of the fused stage-1 matmul as extra rhs columns
  [W | w2 | 0.2*w2 | w1], so no separate scaling pass is needed.

Host-side work is layout/dtype only: slicing, transposition and
fp32->bf16 casts of inputs. All FLOPs happen on device.
"""
import sys

sys.path.insert(0, "/opt/trn_rl_repo")

import numpy as np
import ml_dtypes

import concourse.bass as bass
import concourse.mybir as mybir
from concourse.tile import TileContext
from concourse.bass_utils import run_bass_kernel_spmd

F32 = mybir.dt.float32
BF16 = mybir.dt.bfloat16
F16 = mybir.dt.float16
AF = mybir.ActivationFunctionType
ALU = mybir.AluOpType

ALPHA = 0.2
N_CORES = 8


# ---------------------------------------------------------------------------
# walrus workaround: this build rejects >1 inline sync-wait per instruction
# ("Too many sync wait commands"); move the excess into same-engine NoOps.
# ---------------------------------------------------------------------------
def split_excess_waits(nc, nop_capacity=1):
    counter = 0
    for f in nc.m.functions:
        for bb in f.blocks:
            out = []
            changed = False
            for inst in bb.instructions:
                si = inst.sync_info
                max_inline = 0 if isinstance(inst, mybir.InstDrain) else 1
                if si is not None and len(si.on_wait) > max_inline:
                    waits = list(si.on_wait)
                    if max_inline:
                        excess, keep = waits[:-max_inline], waits[-max_inline:]
                    else:
                        excess, keep = waits, []
                    for s in range(0, len(excess), nop_capacity):
                        counter += 1
                        nop = mybir.InstNoOp(
                            name=f"WSPLIT-{counter}", ins=[], outs=[]
                        )
                        nop.engine = inst.engine
                        nop.sync_info = mybir.SyncInfo(
                            on_wait=excess[s:s + nop_capacity], on_update=[]
                        )
                        out.append(nop)
                    inst.sync_info = mybir.SyncInfo(
                        on_wait=keep, on_update=list(si.on_update)
                    )
                    changed = True
                out.append(inst)
            if changed:
                bb.instructions = out


# ---------------------------------------------------------------------------
# kernel builder
# ---------------------------------------------------------------------------
def build_nc(NJ, R, IN, OUT, acc_banks=6, relu_mod=4, relu_act_lt=0,
             split_waits=True):
    P = 128
    KC = IN // P          # contraction chunks for inp @ W
    JC = NJ // P          # j (column/source-node) chunks
    OC = R // P           # own-row chunks
    G = -(-OC // acc_banks)   # row groups so live accumulators <= acc_banks
    OCG = OC // G
    assert OCG * G == OC
    ISPAN = OCG * P       # free width of the transposed score tiles
    E = OUT + 3           # W | w2 | 0.2*w2 | w1 columns of the fused rhs

    nc = bass.Bass()
    adjT = nc.declare_dram_parameter("adjT", [NJ, R], BF16, isOutput=False)
    # inpT_t: [128, JC*IN] tiled so chunk (jc,kc) is cols jc*IN+kc*P..+P
    inpT_t = nc.declare_dram_parameter(
        "inpT_t", [P, JC * IN], BF16, isOutput=False)
    # own-row version: [128, OC*IN]
    inpTo_t = nc.declare_dram_parameter(
        "inpTo_t", [P, OC * IN], BF16, isOutput=False)
    W = nc.declare_dram_parameter("W", [IN, OUT], BF16, isOutput=False)
    WT = nc.declare_dram_parameter("WT", [OUT, IN], F32, isOutput=False)
    a12 = nc.declare_dram_parameter("a12", [OUT, 2], F32, isOutput=False)
    ident = nc.declare_dram_parameter("ident", [P, P], F32, isOutput=False)
    out_own = nc.declare_dram_parameter("out", [R, OUT], F32, isOutput=True)
    s1_dram = nc.dram_tensor("s1_scratch", [R], F32)

    CC = OUT // P         # chunks of the OUT dim (for W^T @ a12)

    with TileContext(nc) as tc:
        with (
            tc.tile_pool(name="const", bufs=1) as constp,
            tc.tile_pool(name="wts", bufs=1) as wts,
            tc.tile_pool(name="hpool", bufs=1) as hpool,
            tc.tile_pool(name="s1p", bufs=1) as s1p,
            tc.tile_pool(name="inp_t", bufs=3) as inp_p,
            tc.tile_pool(name="adjp", bufs=3) as adjp,
            tc.tile_pool(name="xp", bufs=3) as xp,
            tc.tile_pool(name="eep", bufs=4) as eep,
            tc.tile_pool(name="ptp", bufs=4) as ptp,
            tc.tile_pool(name="ep", bufs=2) as ep,
            tc.tile_pool(name="pmisc", bufs=2, space="PSUM") as pmisc,
            tc.tile_pool(name="pacc", bufs=1, space="PSUM") as pacc,
        ):
            # ---- stage 0: weights ----
            ident_sb = constp.tile([P, P], F32, tag="ident")
            nc.sync.dma_start(out=ident_sb[:, :], in_=ident[:, :])
            wt_sb = []
            a12_sb = []
            for cc in range(CC):
                t = wts.tile([P, IN], F32, tag=f"wt{cc}", name=f"wt{cc}")
                nc.sync.dma_start(out=t[:, :], in_=WT[cc * P:(cc + 1) * P, :])
                wt_sb.append(t)
                t2 = wts.tile([P, 2], F32, tag=f"a12_{cc}", name=f"a12_{cc}")
                nc.sync.dma_start(out=t2[:, :], in_=a12[cc * P:(cc + 1) * P, :])
                a12_sb.append(t2)
            # rhs_cat[kc] = [W rows | w2 | 0.2*w2 | w1]  (bf16, 259 cols)
            rhs_cat = []
            for kc in range(KC):
                t = wts.tile([P, E], BF16, tag=f"rhsc{kc}", name=f"rhsc{kc}")
                nc.sync.dma_start(
                    out=t[:, 0:OUT], in_=W[kc * P:(kc + 1) * P, :]
                )
                rhs_cat.append(t)
            # w12[k, 0:2] = (W @ [a1 a2])[k]  via  WT-blocks^T @ a12-blocks
            for kc in range(KC):
                pw = pmisc.tile([P, E], F32, tag="pm")
                for cc in range(CC):
                    nc.tensor.matmul(
                        pw[:, 0:2],
                        wt_sb[cc][:, kc * P:(kc + 1) * P],
                        a12_sb[cc][:, :],
                        start=(cc == 0),
                        stop=(cc == CC - 1),
                    )
                nc.vector.tensor_copy(
                    rhs_cat[kc][:, OUT:OUT + 1], pw[:, 1:2])
                nc.vector.tensor_scalar_mul(
                    rhs_cat[kc][:, OUT + 1:OUT + 2], pw[:, 1:2], ALPHA)
                nc.vector.tensor_copy(
                    rhs_cat[kc][:, OUT + 2:OUT + 3], pw[:, 0:1])

            # ---- stage 1b: s1 of own rows, then broadcast tile ----
            s1_stage = s1p.tile([P, OC], F32, tag="s1stage")
            for oc in range(OC):
                ito = inp_p.tile([P, IN], BF16, tag="ito", name="ito")
                nc.sync.dma_start(
                    out=ito[:, :],
                    in_=inpTo_t[:, oc * IN:(oc + 1) * IN],
                )
                ps1 = pmisc.tile([P, E], F32, tag="pm")
                for kc in range(KC):
                    nc.tensor.matmul(
                        ps1[:, 0:1],
                        ito[:, kc * P:(kc + 1) * P],
                        rhs_cat[kc][:, OUT + 2:OUT + 3],
                        start=(kc == 0),
                        stop=(kc == KC - 1),
                    )
                nc.vector.tensor_copy(s1_stage[:, oc:oc + 1], ps1[:, 0:1])
            # transpose [128, OC] -> [OC, 128], flatten to DRAM, re-read
            # broadcast across partitions.
            pt1 = pmisc.tile([P, E], F32, tag="pm")
            nc.tensor.matmul(
                pt1[:OC, 0:P], s1_stage[:, :], ident_sb[:, :],
                is_transpose=True,
            )
            s1rows = s1p.tile([P, P], F32, tag="s1rows")
            nc.vector.tensor_copy(s1rows[:OC, :], pt1[:OC, 0:P])
            nc.sync.dma_start(
                out=s1_dram[:].rearrange("(a b) -> a b", b=P),
                in_=s1rows[:OC, :],
            )
            s1bc = s1p.tile([P, R], F32, tag="s1bc")
            nc.sync.dma_start(
                out=s1bc[:, :], in_=s1_dram[:].partition_broadcast(P)
            )
            # fp16 copy: score adds run in fp16 (4x DVE mode); softmax
            # tolerates the rounding (verified ~8e-3 rel err end to end)
            s1bc16 = s1p.tile([P, R], F16, tag="s1bc16")
            nc.vector.tensor_copy(s1bc16[:, :], s1bc[:, :])

            # ---- stages 1+2 fused over g=0; pure stage 2 for g=1 ----
            # stage-1 body (h | s2 | 0.2*s2 for one j-chunk) is emitted
            # inline with the g=0 attention pass so ACT/DVE/PE are all
            # busy from the start instead of a serial h-pass prefix.
            h_sb = [None] * JC
            s2p_sb = [None] * JC

            def stage1_body(jc, it2, a):
                ph = pmisc.tile([P, E], F32, tag="pm")
                for kc in range(KC):
                    nc.tensor.matmul(
                        ph[:, :],
                        it2[:, a * IN + kc * P:a * IN + (kc + 1) * P],
                        rhs_cat[kc][:, :],
                        start=(kc == 0),
                        stop=(kc == KC - 1),
                    )
                h = hpool.tile([P, OUT + 1], BF16, tag=f"h{jc}",
                               name=f"h{jc}")
                nc.scalar.copy(h[:, 0:OUT], ph[:, 0:OUT])
                nc.gpsimd.memset(h[:, OUT:OUT + 1], 1.0)
                # s2 and 0.2*s2 as packed [P,1] per-partition scalars
                sa = hpool.tile([P, 1], F32, tag=f"s2a{jc}", name=f"s2a{jc}")
                sb = hpool.tile([P, 1], F32, tag=f"s2b{jc}", name=f"s2b{jc}")
                nc.vector.tensor_copy(sa[:, 0:1], ph[:, OUT:OUT + 1])
                nc.vector.tensor_copy(sb[:, 0:1], ph[:, OUT + 1:OUT + 2])
                h_sb[jc] = h
                s2p_sb[jc] = (sa, sb)

            def stage2_body(g, gsl, jc, accs):
                at_t = adjp.tile([P, ISPAN], BF16, tag="at")
                nc.sync.dma_start(
                    out=at_t[:, :],
                    in_=adjT[jc * P:(jc + 1) * P, gsl],
                )
                s2, s2b = s2p_sb[jc]
                xq = xp.tile([P, ISPAN], F16, tag="xq")
                if jc % relu_mod < relu_act_lt:
                    # ACT does add+relu (fused bias)
                    nc.scalar.activation(
                        xq[:, :], s1bc16[:, gsl], AF.Relu,
                        bias=s2[:, 0:1], scale=1.0,
                    )
                else:
                    # DVE: x+ = max(s1 + s2, 0) in one tensor_scalar
                    # (fp16 in/out -> 4x mode)
                    nc.vector.tensor_scalar(
                        xq[:, :], s1bc16[:, gsl], s2[:, 0:1], 0.0,
                        op0=ALU.add, op1=ALU.max,
                    )
                e = eep.tile([P, ISPAN], BF16, tag="e")
                nc.scalar.activation(
                    e[:, :], xq[:, :], AF.Exp,
                    bias=s2b[:, 0:1], scale=1.0 - ALPHA,
                )
                pT = ptp.tile([P, ISPAN], BF16, tag="pT")
                nc.vector.tensor_tensor(
                    pT[:, :], e[:, :], at_t[:, :], op=ALU.mult
                )
                rh = h_sb[jc][:, 0:OUT + 1]
                for m in range(OCG):
                    nc.tensor.matmul(
                        accs[m][:, :],
                        pT[:, m * P:(m + 1) * P],
                        rh,
                        start=(jc == 0),
                        stop=(jc == JC - 1),
                    )

            def epilogue(g, accs):
                for m in range(OCG):
                    acc = accs[m]
                    r = ep.tile([P, 1], F32, tag="r")
                    nc.vector.reciprocal(r[:, :], acc[:, OUT:OUT + 1])
                    t = ep.tile([P, OUT], F32, tag="t")
                    nc.vector.tensor_scalar_mul(t[:, :], acc[:, 0:OUT], r[:, :])
                    # elu = relu(t) + (exp(-relu(-t)) - 1)
                    rn = ep.tile([P, OUT], F32, tag="rn")
                    nc.scalar.activation(rn[:, :], t[:, :], AF.Relu, scale=-1.0)
                    en = ep.tile([P, OUT], F32, tag="en")
                    nc.scalar.activation(en[:, :], rn[:, :], AF.Exp, scale=-1.0)
                    ps = ep.tile([P, OUT], F32, tag="ps")
                    nc.scalar.activation(ps[:, :], t[:, :], AF.Relu)
                    res = ep.tile([P, OUT], F32, tag="res")
                    nc.vector.scalar_tensor_tensor(
                        res[:, :], ps[:, :], -1.0, en[:, :],
                        op0=ALU.add, op1=ALU.add,
                    )
                    row = (g * OCG + m) * P
                    nc.sync.dma_start(
                        out=out_own[row:row + P, :], in_=res[:, :]
                    )

            # g=0: fused stage-1 + attention pass
            gsl0 = slice(0, ISPAN)
            accs0 = [
                pacc.tile([P, OUT + 1], F32, tag=f"acc{m}", name=f"acc{m}")
                for m in range(OCG)
            ]
            for jc2 in range(JC // 2):
                it2 = inp_p.tile([P, 2 * IN], BF16, tag="it", name="it")
                nc.sync.dma_start(
                    out=it2[:, :],
                    in_=inpT_t[:, jc2 * 2 * IN:(jc2 + 1) * 2 * IN],
                )
                for a in range(2):
                    jc = jc2 * 2 + a
                    stage1_body(jc, it2, a)
                    stage2_body(0, gsl0, jc, accs0)
            epilogue(0, accs0)

            # remaining g groups: pure attention passes
            for g in range(1, G):
                gsl = slice(g * ISPAN, (g + 1) * ISPAN)
                accs = [
                    pacc.tile([P, OUT + 1], F32, tag=f"acc{m}", name=f"acc{m}")
                    for m in range(OCG)
                ]
                for jc in range(JC):
                    stage2_body(g, gsl, jc, accs)
                epilogue(g, accs)

    if split_waits:
        split_excess_waits(nc)
    return nc


# ---------------------------------------------------------------------------
# host wrapper
# ---------------------------------------------------------------------------
_CACHE = {}


def _get_nc(NJ, R, IN, OUT):
    key = (NJ, R, IN, OUT)
    if key not in _CACHE:
        _CACHE[key] = build_nc(NJ, R, IN, OUT)
    return _CACHE[key]


def _tile_inpT(inp_bf16, P=128):
    """[Nrows, IN] -> [128, (Nrows/P)*IN] where chunk (jc,kc) at cols
    jc*IN+kc*P..+P holds inpT[kc*P+p, jc*P+t] = inp[jc*P+t, kc*P+p]."""
    Nr, IN = inp_bf16.shape
    JC, KC = Nr // P, IN // P
    a = inp_bf16.reshape(JC, P, KC, P)        # [jc, t, kc, p]
    a = np.ascontiguousarray(a.transpose(3, 0, 2, 1))  # [p, jc, kc, t]
    return a.reshape(P, JC * IN)


def prep_in_maps(inp, adj, W, a1, a2, n_cores=N_CORES):
    """Host-side layout prep: slicing + transposition + dtype casts only."""
    N, IN = inp.shape
    OUT = W.shape[1]
    R = N // n_cores
    bf16 = ml_dtypes.bfloat16
    inp_bf = inp.astype(bf16)
    inpT_t = _tile_inpT(inp_bf)
    W_bf = np.ascontiguousarray(W.astype(bf16))
    WT = np.ascontiguousarray(W.T)
    a12 = np.ascontiguousarray(np.concatenate([a1, a2], axis=1))
    ident = np.eye(128, dtype=np.float32)
    adj_bf = adj.astype(bf16)
    in_maps = []
    for c in range(n_cores):
        sl = slice(c * R, (c + 1) * R)
        in_maps.append({
            "adjT": np.ascontiguousarray(adj_bf[sl, :].T),
            "inpT_t": inpT_t,
            "inpTo_t": _tile_inpT(inp_bf[sl, :]),
            "W": W_bf,
            "WT": WT,
            "a12": a12,
            "ident": ident,
        })
    return in_maps, R, IN, OUT


def kernel(inp, adj, W, a1, a2):
    inp = np.asarray(inp, dtype=np.float32)
    adj = np.asarray(adj, dtype=np.int32)
    W = np.asarray(W, dtype=np.float32)
    a1 = np.asarray(a1, dtype=np.float32)
    a2 = np.asarray(a2, dtype=np.float32)
    N = inp.shape[0]
    in_maps, R, IN, OUT = prep_in_maps(inp, adj, W, a1, a2)
    nc = _get_nc(N, R, IN, OUT)
    res = run_bass_kernel_spmd(nc, in_maps, list(range(N_CORES)))
    return np.concatenate(
        [res.results[c]["out"] for c in range(N_CORES)], axis=0
    )


# revision 24
# speedup vs baseline: 1.1107x; 1.1107x over previous
"""GAT attention layer (gnn_message_passing) on 8 TRN2 NeuronCores.

Math (reference):
    h = inp @ W;  s1 = h @ a1;  s2 = h @ a2
    e = leaky_relu(s1 + s2^T, 0.2);  scores = where(adj>0, e, -9e15)
    out = elu(softmax_row(scores) @ h)

Device algorithm (per core, rows R = N/8):
  Softmax without max-subtraction; masked entries contribute exactly 0 via
  P = adj * exp(e):
      out[i,:] = elu( (sum_j P[i,j] h[j,:]) / (sum_j P[i,j]) )
  Key identity:  exp(lrelu(x)) = exp(0.2*s1) * exp(0.2*s2) * exp(0.8*relu(x))
  with x = s1[i]+s2[j].  The exp(0.2*s1[i]) factor is constant along the
  softmax row, so it cancels between numerator and denominator and is
  simply dropped.  The 0.2*s2[j] term is per-partition and rides the
  activation bias slot.  Per element this leaves:
      x+ = max(s1[i] + s2[j], 0)        [DVE tensor_scalar or ACT Relu]
      e  = Exp(0.8*x+ + 0.2*s2[j])      [ACT, bf16 out]
      pT = e * adjT                     [DVE tensor_tensor, bf16]
  Everything is in the TRANSPOSED orientation [j (partitions), i (free)]
  so the attention matmul needs no on-chip transposes.  num|denom in one
  bf16 matmul: rhs = [h | ones] (257 cols), lhsT = P^T slices.
  s2 and 0.2*s2 come out of the fused stage-1 matmul as extra rhs columns
  [W | w2 | 0.2*w2 | w1], so no separate scaling pass is needed.

Host-side work is layout/dtype only: slicing, transposition and
fp32->bf16 casts of inputs. All FLOPs happen on device.
"""
import sys

sys.path.insert(0, "/opt/trn_rl_repo")

import numpy as np
import ml_dtypes

import concourse.bass as bass
import concourse.mybir as mybir
from concourse.tile import TileContext
from concourse.bass_utils import run_bass_kernel_spmd

F32 = mybir.dt.float32
BF16 = mybir.dt.bfloat16
F16 = mybir.dt.float16
AF = mybir.ActivationFunctionType
ALU = mybir.AluOpType

ALPHA = 0.2
N_CORES = 8


# ---------------------------------------------------------------------------
# walrus workaround: this build rejects >1 inline sync-wait per instruction
# ("Too many sync wait commands"); move the excess into same-engine NoOps.
# ---------------------------------------------------------------------------
def split_excess_waits(nc, nop_capacity=1):
    counter = 0
    for f in nc.m.functions:
        for bb in f.blocks:
            out = []
            changed = False
            for inst in bb.instructions:
                si = inst.sync_info
                max_inline = 0 if isinstance(inst, mybir.InstDrain) else 1
                if si is not None and len(si.on_wait) > max_inline:
                    waits = list(si.on_wait)
                    if max_inline:
                        excess, keep = waits[:-max_inline], waits[-max_inline:]
                    else:
                        excess, keep = waits, []
                    for s in range(0, len(excess), nop_capacity):
                        counter += 1
                        nop = mybir.InstNoOp(
                            name=f"WSPLIT-{counter}", ins=[], outs=[]
                        )
                        nop.engine = inst.engine
                        nop.sync_info = mybir.SyncInfo(
                            on_wait=excess[s:s + nop_capacity], on_update=[]
                        )
                        out.append(nop)
                    inst.sync_info = mybir.SyncInfo(
                        on_wait=keep, on_update=list(si.on_update)
                    )
                    changed = True
                out.append(inst)
            if changed:
                bb.instructions = out


# ---------------------------------------------------------------------------
# kernel builder
# ---------------------------------------------------------------------------
def build_nc(NJ, R, IN, OUT, acc_banks=6, relu_mod=4, relu_act_lt=0,
             interleave=True, hcopy_dve_mod=0, adj_fp8=False,
             epi_dve=True, scale_fold=False, bufs_adj=6, bufs_e=6,
             bufs_x=3, split_waits=True):
    P = 128
    KC = IN // P          # contraction chunks for inp @ W
    JC = NJ // P          # j (column/source-node) chunks
    OC = R // P           # own-row chunks
    G = -(-OC // acc_banks)   # row groups so live accumulators <= acc_banks
    OCG = OC // G
    assert OCG * G == OC
    ISPAN = OCG * P       # free width of the transposed score tiles
    E = OUT + 3           # W | w2 | 0.2*w2 | w1 columns of the fused rhs

    nc = bass.Bass()
    F8 = mybir.dt.float8e4
    adjT = nc.declare_dram_parameter(
        "adjT", [NJ, R], F8 if adj_fp8 else BF16, isOutput=False)
    # inpT_t: [128, JC*IN] tiled so chunk (jc,kc) is cols jc*IN+kc*P..+P
    inpT_t = nc.declare_dram_parameter(
        "inpT_t", [P, JC * IN], BF16, isOutput=False)
    # own-row version: [128, OC*IN]
    inpTo_t = nc.declare_dram_parameter(
        "inpTo_t", [P, OC * IN], BF16, isOutput=False)
    W = nc.declare_dram_parameter("W", [IN, OUT], BF16, isOutput=False)
    WT = nc.declare_dram_parameter("WT", [OUT, IN], F32, isOutput=False)
    a12 = nc.declare_dram_parameter("a12", [OUT, 2], F32, isOutput=False)
    ident = nc.declare_dram_parameter("ident", [P, P], F32, isOutput=False)
    out_own = nc.declare_dram_parameter("out", [R, OUT], F32, isOutput=True)
    s1_dram = nc.dram_tensor("s1_scratch", [R], F32)

    CC = OUT // P         # chunks of the OUT dim (for W^T @ a12)

    with TileContext(nc) as tc:
        with (
            tc.tile_pool(name="const", bufs=1) as constp,
            tc.tile_pool(name="wts", bufs=1) as wts,
            tc.tile_pool(name="hpool", bufs=1) as hpool,
            tc.tile_pool(name="s1p", bufs=1) as s1p,
            tc.tile_pool(name="inp_t", bufs=3) as inp_p,
            tc.tile_pool(name="adjp", bufs=bufs_adj) as adjp,
            tc.tile_pool(name="xp", bufs=bufs_x) as xp,
            tc.tile_pool(name="eep", bufs=bufs_e) as eep,
            tc.tile_pool(name="ptp", bufs=bufs_e) as ptp,
            tc.tile_pool(name="ep", bufs=2) as ep,
            tc.tile_pool(name="pmisc", bufs=2, space="PSUM") as pmisc,
            tc.tile_pool(name="pacc", bufs=1, space="PSUM") as pacc,
        ):
            # ---- stage 0: weights ----
            ident_sb = constp.tile([P, P], F32, tag="ident")
            nc.sync.dma_start(out=ident_sb[:, :], in_=ident[:, :])
            wt_sb = []
            a12_sb = []
            for cc in range(CC):
                t = wts.tile([P, IN], F32, tag=f"wt{cc}", name=f"wt{cc}")
                nc.sync.dma_start(out=t[:, :], in_=WT[cc * P:(cc + 1) * P, :])
                wt_sb.append(t)
                t2 = wts.tile([P, 2], F32, tag=f"a12_{cc}", name=f"a12_{cc}")
                nc.sync.dma_start(out=t2[:, :], in_=a12[cc * P:(cc + 1) * P, :])
                a12_sb.append(t2)
            # rhs_cat[kc] = [W rows | w2 | 0.2*w2 | w1]  (bf16, 259 cols)
            rhs_cat = []
            for kc in range(KC):
                t = wts.tile([P, E], BF16, tag=f"rhsc{kc}", name=f"rhsc{kc}")
                nc.sync.dma_start(
                    out=t[:, 0:OUT], in_=W[kc * P:(kc + 1) * P, :]
                )
                rhs_cat.append(t)
            # w12[k, 0:2] = (W @ [a1 a2])[k]  via  WT-blocks^T @ a12-blocks
            for kc in range(KC):
                pw = pmisc.tile([P, E], F32, tag="pm")
                for cc in range(CC):
                    nc.tensor.matmul(
                        pw[:, 0:2],
                        wt_sb[cc][:, kc * P:(kc + 1) * P],
                        a12_sb[cc][:, :],
                        start=(cc == 0),
                        stop=(cc == CC - 1),
                    )
                if scale_fold:
                    nc.vector.tensor_scalar_mul(
                        rhs_cat[kc][:, OUT:OUT + 1], pw[:, 1:2], 1.0 - ALPHA)
                else:
                    nc.vector.tensor_copy(
                        rhs_cat[kc][:, OUT:OUT + 1], pw[:, 1:2])
                nc.vector.tensor_scalar_mul(
                    rhs_cat[kc][:, OUT + 1:OUT + 2], pw[:, 1:2], ALPHA)
                nc.vector.tensor_copy(
                    rhs_cat[kc][:, OUT + 2:OUT + 3], pw[:, 0:1])

            # ---- stage 1b: s1 of own rows, then broadcast tile ----
            s1_stage = s1p.tile([P, OC], F32, tag="s1stage")
            for oc in range(OC):
                ito = inp_p.tile([P, IN], BF16, tag="ito", name="ito")
                nc.sync.dma_start(
                    out=ito[:, :],
                    in_=inpTo_t[:, oc * IN:(oc + 1) * IN],
                )
                ps1 = pmisc.tile([P, E], F32, tag="pm")
                for kc in range(KC):
                    nc.tensor.matmul(
                        ps1[:, 0:1],
                        ito[:, kc * P:(kc + 1) * P],
                        rhs_cat[kc][:, OUT + 2:OUT + 3],
                        start=(kc == 0),
                        stop=(kc == KC - 1),
                    )
                nc.vector.tensor_copy(s1_stage[:, oc:oc + 1], ps1[:, 0:1])
            # transpose [128, OC] -> [OC, 128], flatten to DRAM, re-read
            # broadcast across partitions.
            pt1 = pmisc.tile([P, E], F32, tag="pm")
            nc.tensor.matmul(
                pt1[:OC, 0:P], s1_stage[:, :], ident_sb[:, :],
                is_transpose=True,
            )
            s1rows = s1p.tile([P, P], F32, tag="s1rows")
            nc.vector.tensor_copy(s1rows[:OC, :], pt1[:OC, 0:P])
            nc.sync.dma_start(
                out=s1_dram[:].rearrange("(a b) -> a b", b=P),
                in_=s1rows[:OC, :],
            )
            s1bc = s1p.tile([P, R], F32, tag="s1bc")
            nc.sync.dma_start(
                out=s1bc[:, :], in_=s1_dram[:].partition_broadcast(P)
            )
            # fp16 copy: score adds run in fp16 (4x DVE mode); softmax
            # tolerates the rounding (verified ~8e-3 rel err end to end)
            s1bc16 = s1p.tile([P, R], F16, tag="s1bc16")
            if scale_fold:
                nc.vector.tensor_scalar_mul(
                    s1bc16[:, :], s1bc[:, :], 1.0 - ALPHA)
            else:
                nc.vector.tensor_copy(s1bc16[:, :], s1bc[:, :])

            # ---- stages 1+2 fused over g=0; pure stage 2 for g=1 ----
            # stage-1 body (h | s2 | 0.2*s2 for one j-chunk) is emitted
            # inline with the g=0 attention pass so ACT/DVE/PE are all
            # busy from the start instead of a serial h-pass prefix.
            h_sb = [None] * JC
            s2p_sb = [None] * JC

            def stage1_body(jc, it2, a):
                ph = pmisc.tile([P, E], F32, tag="pm")
                for kc in range(KC):
                    nc.tensor.matmul(
                        ph[:, :],
                        it2[:, a * IN + kc * P:a * IN + (kc + 1) * P],
                        rhs_cat[kc][:, :],
                        start=(kc == 0),
                        stop=(kc == KC - 1),
                    )
                h = hpool.tile([P, OUT + 1], BF16, tag=f"h{jc}",
                               name=f"h{jc}")
                if hcopy_dve_mod and jc % hcopy_dve_mod == 0:
                    nc.vector.tensor_copy(h[:, 0:OUT], ph[:, 0:OUT])
                else:
                    nc.scalar.copy(h[:, 0:OUT], ph[:, 0:OUT])
                nc.gpsimd.memset(h[:, OUT:OUT + 1], 1.0)
                # s2 and 0.2*s2 as packed [P,1] per-partition scalars
                sa = hpool.tile([P, 1], F32, tag=f"s2a{jc}", name=f"s2a{jc}")
                sb = hpool.tile([P, 1], F32, tag=f"s2b{jc}", name=f"s2b{jc}")
                nc.vector.tensor_copy(sa[:, 0:1], ph[:, OUT:OUT + 1])
                nc.vector.tensor_copy(sb[:, 0:1], ph[:, OUT + 1:OUT + 2])
                h_sb[jc] = h
                s2p_sb[jc] = (sa, sb)

            def stage2_body(g, gsl, jc, accs):
                at_t = adjp.tile([P, ISPAN], BF16, tag="at")
                if adj_fp8:
                    nc.gpsimd.dma_start(
                        out=at_t[:, :],
                        in_=adjT[jc * P:(jc + 1) * P, gsl],
                    )
                else:
                    nc.sync.dma_start(
                        out=at_t[:, :],
                        in_=adjT[jc * P:(jc + 1) * P, gsl],
                    )
                s2, s2b = s2p_sb[jc]
                xq = xp.tile([P, ISPAN], F16, tag="xq")
                if jc % relu_mod < relu_act_lt:
                    # ACT does add+relu (fused bias)
                    nc.scalar.activation(
                        xq[:, :], s1bc16[:, gsl], AF.Relu,
                        bias=s2[:, 0:1], scale=1.0,
                    )
                else:
                    # DVE: x+ = max(s1 + s2, 0) in one tensor_scalar
                    # (fp16 in/out -> 4x mode)
                    nc.vector.tensor_scalar(
                        xq[:, :], s1bc16[:, gsl], s2[:, 0:1], 0.0,
                        op0=ALU.add, op1=ALU.max,
                    )
                e = eep.tile([P, ISPAN], BF16, tag="e")
                nc.scalar.activation(
                    e[:, :], xq[:, :], AF.Exp,
                    bias=s2b[:, 0:1],
                    scale=1.0 if scale_fold else 1.0 - ALPHA,
                )
                pT = ptp.tile([P, ISPAN], BF16, tag="pT")
                nc.vector.tensor_tensor(
                    pT[:, :], e[:, :], at_t[:, :], op=ALU.mult
                )
                rh = h_sb[jc][:, 0:OUT + 1]
                for m in range(OCG):
                    nc.tensor.matmul(
                        accs[m][:, :],
                        pT[:, m * P:(m + 1) * P],
                        rh,
                        start=(jc == 0),
                        stop=(jc == JC - 1),
                    )

            def epilogue(g, accs):
                for m in range(OCG):
                    acc = accs[m]
                    r = ep.tile([P, 1], F32, tag="r")
                    nc.vector.reciprocal(r[:, :], acc[:, OUT:OUT + 1])
                    t = ep.tile([P, OUT], F32, tag="t")
                    nc.vector.tensor_scalar_mul(t[:, :], acc[:, 0:OUT], r[:, :])
                    # elu = relu(t) + (exp(-relu(-t)) - 1)
                    rn = ep.tile([P, OUT], F32, tag="rn")
                    en = ep.tile([P, OUT], F32, tag="en")
                    ps = ep.tile([P, OUT], F32, tag="ps")
                    if epi_dve:
                        nc.vector.tensor_scalar(
                            rn[:, :], t[:, :], -1.0, 0.0,
                            op0=ALU.mult, op1=ALU.max)
                        nc.scalar.activation(
                            en[:, :], rn[:, :], AF.Exp, scale=-1.0)
                        nc.vector.tensor_scalar_max(ps[:, :], t[:, :], 0.0)
                    else:
                        nc.scalar.activation(
                            rn[:, :], t[:, :], AF.Relu, scale=-1.0)
                        nc.scalar.activation(
                            en[:, :], rn[:, :], AF.Exp, scale=-1.0)
                        nc.scalar.activation(ps[:, :], t[:, :], AF.Relu)
                    res = ep.tile([P, OUT], F32, tag="res")
                    nc.vector.scalar_tensor_tensor(
                        res[:, :], ps[:, :], -1.0, en[:, :],
                        op0=ALU.add, op1=ALU.add,
                    )
                    row = (g * OCG + m) * P
                    nc.sync.dma_start(
                        out=out_own[row:row + P, :], in_=res[:, :]
                    )

            if interleave:
                # g=0: fused stage-1 + attention pass
                gsl0 = slice(0, ISPAN)
                accs0 = [
                    pacc.tile([P, OUT + 1], F32, tag=f"acc{m}",
                              name=f"acc{m}")
                    for m in range(OCG)
                ]
                for jc2 in range(JC // 2):
                    it2 = inp_p.tile([P, 2 * IN], BF16, tag="it", name="it")
                    nc.sync.dma_start(
                        out=it2[:, :],
                        in_=inpT_t[:, jc2 * 2 * IN:(jc2 + 1) * 2 * IN],
                    )
                    for a in range(2):
                        jc = jc2 * 2 + a
                        stage1_body(jc, it2, a)
                        stage2_body(0, gsl0, jc, accs0)
                epilogue(0, accs0)
                g_start = 1
            else:
                # plain stage 1 first
                for jc2 in range(JC // 2):
                    it2 = inp_p.tile([P, 2 * IN], BF16, tag="it", name="it")
                    nc.sync.dma_start(
                        out=it2[:, :],
                        in_=inpT_t[:, jc2 * 2 * IN:(jc2 + 1) * 2 * IN],
                    )
                    for a in range(2):
                        stage1_body(jc2 * 2 + a, it2, a)
                g_start = 0

            # remaining g groups: pure attention passes
            for g in range(g_start, G):
                gsl = slice(g * ISPAN, (g + 1) * ISPAN)
                accs = [
                    pacc.tile([P, OUT + 1], F32, tag=f"acc{m}", name=f"acc{m}")
                    for m in range(OCG)
                ]
                for jc in range(JC):
                    stage2_body(g, gsl, jc, accs)
                epilogue(g, accs)

    if split_waits:
        split_excess_waits(nc)
    return nc


# ---------------------------------------------------------------------------
# host wrapper
# ---------------------------------------------------------------------------
_CACHE = {}


def _get_nc(NJ, R, IN, OUT):
    key = (NJ, R, IN, OUT)
    if key not in _CACHE:
        _CACHE[key] = build_nc(NJ, R, IN, OUT)
    return _CACHE[key]


def _tile_inpT(inp_bf16, P=128):
    """[Nrows, IN] -> [128, (Nrows/P)*IN] where chunk (jc,kc) at cols
    jc*IN+kc*P..+P holds inpT[kc*P+p, jc*P+t] = inp[jc*P+t, kc*P+p]."""
    Nr, IN = inp_bf16.shape
    JC, KC = Nr // P, IN // P
    a = inp_bf16.reshape(JC, P, KC, P)        # [jc, t, kc, p]
    a = np.ascontiguousarray(a.transpose(3, 0, 2, 1))  # [p, jc, kc, t]
    return a.reshape(P, JC * IN)


def prep_in_maps(inp, adj, W, a1, a2, n_cores=N_CORES, adj_fp8=False):
    """Host-side layout prep: slicing + transposition + dtype casts only."""
    N, IN = inp.shape
    OUT = W.shape[1]
    R = N // n_cores
    bf16 = ml_dtypes.bfloat16
    inp_bf = inp.astype(bf16)
    inpT_t = _tile_inpT(inp_bf)
    W_bf = np.ascontiguousarray(W.astype(bf16))
    WT = np.ascontiguousarray(W.T)
    a12 = np.ascontiguousarray(np.concatenate([a1, a2], axis=1))
    adj_bf = adj.astype(ml_dtypes.float8_e4m3 if adj_fp8 else bf16)
    ident = np.eye(128, dtype=np.float32)
    in_maps = []
    for c in range(n_cores):
        sl = slice(c * R, (c + 1) * R)
        in_maps.append({
            "adjT": np.ascontiguousarray(adj_bf[sl, :].T),
            "inpT_t": inpT_t,
            "inpTo_t": _tile_inpT(inp_bf[sl, :]),
            "W": W_bf,
            "WT": WT,
            "a12": a12,
            "ident": ident,
        })
    return in_maps, R, IN, OUT


def kernel(inp, adj, W, a1, a2):
    inp = np.asarray(inp, dtype=np.float32)
    adj = np.asarray(adj, dtype=np.int32)
    W = np.asarray(W, dtype=np.float32)
    a1 = np.asarray(a1, dtype=np.float32)
    a2 = np.asarray(a2, dtype=np.float32)
    N = inp.shape[0]
    in_maps, R, IN, OUT = prep_in_maps(inp, adj, W, a1, a2)
    nc = _get_nc(N, R, IN, OUT)
    res = run_bass_kernel_spmd(nc, in_maps, list(range(N_CORES)))
    return np.concatenate(
        [res.results[c]["out"] for c in range(N_CORES)], axis=0
    )


# revision 26
# speedup vs baseline: 1.3936x; 1.2547x over previous
"""GAT attention layer (gnn_message_passing) on 8 TRN2 NeuronCores.

Math (reference):
    h = inp @ W;  s1 = h @ a1;  s2 = h @ a2
    e = leaky_relu(s1 + s2^T, 0.2);  scores = where(adj>0, e, -9e15)
    out = elu(softmax_row(scores) @ h)

Device algorithm (per core, rows R = N/8):
  Softmax without max-subtraction; masked entries contribute exactly 0 via
  P = adj * exp(e):
      out[i,:] = elu( (sum_j P[i,j] h[j,:]) / (sum_j P[i,j]) )
  Key identity:  exp(lrelu(x)) = exp(0.2*s1) * exp(0.2*s2) * exp(0.8*relu(x))
  with x = s1[i]+s2[j].  The exp(0.2*s1[i]) factor is constant along the
  softmax row, so it cancels between numerator and denominator and is
  simply dropped.  The 0.2*s2[j] term is per-partition and rides the
  activation bias slot.  Per element this leaves:
      x+ = max(s1[i] + s2[j], 0)        [DVE tensor_scalar or ACT Relu]
      e  = Exp(0.8*x+ + 0.2*s2[j])      [ACT, bf16 out]
      pT = e * adjT                     [DVE tensor_tensor, bf16]
  Everything is in the TRANSPOSED orientation [j (partitions), i (free)]
  so the attention matmul needs no on-chip transposes.  num|denom in one
  bf16 matmul: rhs = [h | ones] (257 cols), lhsT = P^T slices.
  s2 and 0.2*s2 come out of the fused stage-1 matmul as extra rhs columns
  [W | w2 | 0.2*w2 | w1], so no separate scaling pass is needed.

Host-side work is layout/dtype only: slicing, transposition and
fp32->bf16 casts of inputs. All FLOPs happen on device.
"""
import sys

sys.path.insert(0, "/opt/trn_rl_repo")

import numpy as np
import ml_dtypes

import concourse.bass as bass
import concourse.mybir as mybir
from concourse.tile import TileContext
from concourse.bass_utils import run_bass_kernel_spmd

F32 = mybir.dt.float32
BF16 = mybir.dt.bfloat16
F16 = mybir.dt.float16
AF = mybir.ActivationFunctionType
ALU = mybir.AluOpType

ALPHA = 0.2
N_CORES = 8


# ---------------------------------------------------------------------------
# walrus workaround: this build rejects >1 inline sync-wait per instruction
# ("Too many sync wait commands"); move the excess into same-engine NoOps.
# ---------------------------------------------------------------------------
def split_excess_waits(nc, nop_capacity=1):
    counter = 0
    for f in nc.m.functions:
        for bb in f.blocks:
            out = []
            changed = False
            for inst in bb.instructions:
                si = inst.sync_info
                max_inline = 0 if isinstance(inst, mybir.InstDrain) else 1
                if si is not None and len(si.on_wait) > max_inline:
                    waits = list(si.on_wait)
                    if max_inline:
                        excess, keep = waits[:-max_inline], waits[-max_inline:]
                    else:
                        excess, keep = waits, []
                    for s in range(0, len(excess), nop_capacity):
                        counter += 1
                        nop = mybir.InstNoOp(
                            name=f"WSPLIT-{counter}", ins=[], outs=[]
                        )
                        nop.engine = inst.engine
                        nop.sync_info = mybir.SyncInfo(
                            on_wait=excess[s:s + nop_capacity], on_update=[]
                        )
                        out.append(nop)
                    inst.sync_info = mybir.SyncInfo(
                        on_wait=keep, on_update=list(si.on_update)
                    )
                    changed = True
                out.append(inst)
            if changed:
                bb.instructions = out


# ---------------------------------------------------------------------------
# kernel builder
# ---------------------------------------------------------------------------
def build_nc(NJ, R, IN, OUT, acc_banks=6, relu_mod=4, relu_act_lt=0,
             interleave=True, hcopy_dve_mod=0, adj_fp8=False,
             epi_dve=True, scale_fold=False, bufs_adj=6, bufs_e=6,
             bufs_x=3, lead2=2, split_waits=True):
    P = 128
    KC = IN // P          # contraction chunks for inp @ W
    JC = NJ // P          # j (column/source-node) chunks
    OC = R // P           # own-row chunks
    G = -(-OC // acc_banks)   # row groups so live accumulators <= acc_banks
    OCG = OC // G
    assert OCG * G == OC
    ISPAN = OCG * P       # free width of the transposed score tiles
    E = OUT + 3           # W | w2 | 0.2*w2 | w1 columns of the fused rhs

    nc = bass.Bass()
    F8 = mybir.dt.float8e4
    adjT = nc.declare_dram_parameter(
        "adjT", [NJ, R], F8 if adj_fp8 else BF16, isOutput=False)
    # inpT_t: [128, JC*IN] tiled so chunk (jc,kc) is cols jc*IN+kc*P..+P
    inpT_t = nc.declare_dram_parameter(
        "inpT_t", [P, JC * IN], BF16, isOutput=False)
    # own-row version: [128, OC*IN]
    inpTo_t = nc.declare_dram_parameter(
        "inpTo_t", [P, OC * IN], BF16, isOutput=False)
    W = nc.declare_dram_parameter("W", [IN, OUT], BF16, isOutput=False)
    WT = nc.declare_dram_parameter("WT", [OUT, IN], F32, isOutput=False)
    a12 = nc.declare_dram_parameter("a12", [OUT, 2], F32, isOutput=False)
    ident = nc.declare_dram_parameter("ident", [P, P], F32, isOutput=False)
    out_own = nc.declare_dram_parameter("out", [R, OUT], F32, isOutput=True)
    s1_dram = nc.dram_tensor("s1_scratch", [R], F32)

    CC = OUT // P         # chunks of the OUT dim (for W^T @ a12)

    with TileContext(nc) as tc:
        with (
            tc.tile_pool(name="const", bufs=1) as constp,
            tc.tile_pool(name="wts", bufs=1) as wts,
            tc.tile_pool(name="hpool", bufs=1) as hpool,
            tc.tile_pool(name="s1p", bufs=1) as s1p,
            tc.tile_pool(name="inp_t", bufs=3) as inp_p,
            tc.tile_pool(name="adjp", bufs=bufs_adj) as adjp,
            tc.tile_pool(name="xp", bufs=bufs_x) as xp,
            tc.tile_pool(name="eep", bufs=bufs_e) as eep,
            tc.tile_pool(name="ptp", bufs=bufs_e) as ptp,
            tc.tile_pool(name="ep", bufs=2) as ep,
            tc.tile_pool(name="pmisc", bufs=2, space="PSUM") as pmisc,
            tc.tile_pool(name="pacc", bufs=1, space="PSUM") as pacc,
        ):
            # ---- stage 0: weights ----
            ident_sb = constp.tile([P, P], F32, tag="ident")
            nc.sync.dma_start(out=ident_sb[:, :], in_=ident[:, :])
            wt_sb = []
            a12_sb = []
            for cc in range(CC):
                t = wts.tile([P, IN], F32, tag=f"wt{cc}", name=f"wt{cc}")
                nc.sync.dma_start(out=t[:, :], in_=WT[cc * P:(cc + 1) * P, :])
                wt_sb.append(t)
                t2 = wts.tile([P, 2], F32, tag=f"a12_{cc}", name=f"a12_{cc}")
                nc.sync.dma_start(out=t2[:, :], in_=a12[cc * P:(cc + 1) * P, :])
                a12_sb.append(t2)
            # rhs_cat[kc] = [W rows | w2 | 0.2*w2 | w1]  (bf16, 259 cols)
            rhs_cat = []
            for kc in range(KC):
                t = wts.tile([P, E], BF16, tag=f"rhsc{kc}", name=f"rhsc{kc}")
                nc.sync.dma_start(
                    out=t[:, 0:OUT], in_=W[kc * P:(kc + 1) * P, :]
                )
                rhs_cat.append(t)
            # w12[k, 0:2] = (W @ [a1 a2])[k]  via  WT-blocks^T @ a12-blocks
            for kc in range(KC):
                pw = pmisc.tile([P, E], F32, tag="pm")
                for cc in range(CC):
                    nc.tensor.matmul(
                        pw[:, 0:2],
                        wt_sb[cc][:, kc * P:(kc + 1) * P],
                        a12_sb[cc][:, :],
                        start=(cc == 0),
                        stop=(cc == CC - 1),
                    )
                if scale_fold:
                    nc.vector.tensor_scalar_mul(
                        rhs_cat[kc][:, OUT:OUT + 1], pw[:, 1:2], 1.0 - ALPHA)
                else:
                    nc.vector.tensor_copy(
                        rhs_cat[kc][:, OUT:OUT + 1], pw[:, 1:2])
                nc.vector.tensor_scalar_mul(
                    rhs_cat[kc][:, OUT + 1:OUT + 2], pw[:, 1:2], ALPHA)
                nc.vector.tensor_copy(
                    rhs_cat[kc][:, OUT + 2:OUT + 3], pw[:, 0:1])

            # ---- stage 1b: s1 of own rows, then broadcast tile ----
            s1_stage = s1p.tile([P, OC], F32, tag="s1stage")
            for oc in range(OC):
                ito = inp_p.tile([P, IN], BF16, tag="ito", name="ito")
                nc.sync.dma_start(
                    out=ito[:, :],
                    in_=inpTo_t[:, oc * IN:(oc + 1) * IN],
                )
                ps1 = pmisc.tile([P, E], F32, tag="pm")
                for kc in range(KC):
                    nc.tensor.matmul(
                        ps1[:, 0:1],
                        ito[:, kc * P:(kc + 1) * P],
                        rhs_cat[kc][:, OUT + 2:OUT + 3],
                        start=(kc == 0),
                        stop=(kc == KC - 1),
                    )
                nc.vector.tensor_copy(s1_stage[:, oc:oc + 1], ps1[:, 0:1])
            # transpose [128, OC] -> [OC, 128], flatten to DRAM, re-read
            # broadcast across partitions.
            pt1 = pmisc.tile([P, E], F32, tag="pm")
            nc.tensor.matmul(
                pt1[:OC, 0:P], s1_stage[:, :], ident_sb[:, :],
                is_transpose=True,
            )
            s1rows = s1p.tile([P, P], F32, tag="s1rows")
            nc.vector.tensor_copy(s1rows[:OC, :], pt1[:OC, 0:P])
            nc.sync.dma_start(
                out=s1_dram[:].rearrange("(a b) -> a b", b=P),
                in_=s1rows[:OC, :],
            )
            s1bc = s1p.tile([P, R], F32, tag="s1bc")
            nc.sync.dma_start(
                out=s1bc[:, :], in_=s1_dram[:].partition_broadcast(P)
            )
            # fp16 copy: score adds run in fp16 (4x DVE mode); softmax
            # tolerates the rounding (verified ~8e-3 rel err end to end)
            s1bc16 = s1p.tile([P, R], F16, tag="s1bc16")
            if scale_fold:
                nc.vector.tensor_scalar_mul(
                    s1bc16[:, :], s1bc[:, :], 1.0 - ALPHA)
            else:
                nc.vector.tensor_copy(s1bc16[:, :], s1bc[:, :])

            # ---- stages 1+2 fused over g=0; pure stage 2 for g=1 ----
            # stage-1 body (h | s2 | 0.2*s2 for one j-chunk) is emitted
            # inline with the g=0 attention pass so ACT/DVE/PE are all
            # busy from the start instead of a serial h-pass prefix.
            h_sb = [None] * JC
            s2p_sb = [None] * JC

            def stage1_body(jc, it2, a):
                ph = pmisc.tile([P, E], F32, tag="pm")
                for kc in range(KC):
                    nc.tensor.matmul(
                        ph[:, :],
                        it2[:, a * IN + kc * P:a * IN + (kc + 1) * P],
                        rhs_cat[kc][:, :],
                        start=(kc == 0),
                        stop=(kc == KC - 1),
                    )
                h = hpool.tile([P, OUT + 1], BF16, tag=f"h{jc}",
                               name=f"h{jc}")
                if hcopy_dve_mod and jc % hcopy_dve_mod == 0:
                    nc.vector.tensor_copy(h[:, 0:OUT], ph[:, 0:OUT])
                else:
                    nc.scalar.copy(h[:, 0:OUT], ph[:, 0:OUT])
                nc.gpsimd.memset(h[:, OUT:OUT + 1], 1.0)
                # s2 and 0.2*s2 as packed [P,1] per-partition scalars
                sa = hpool.tile([P, 1], F32, tag=f"s2a{jc}", name=f"s2a{jc}")
                sb = hpool.tile([P, 1], F32, tag=f"s2b{jc}", name=f"s2b{jc}")
                nc.vector.tensor_copy(sa[:, 0:1], ph[:, OUT:OUT + 1])
                nc.vector.tensor_copy(sb[:, 0:1], ph[:, OUT + 1:OUT + 2])
                h_sb[jc] = h
                s2p_sb[jc] = (sa, sb)

            def stage2_body(g, gsl, jc, accs):
                at_t = adjp.tile([P, ISPAN], BF16, tag="at")
                if adj_fp8:
                    nc.gpsimd.dma_start(
                        out=at_t[:, :],
                        in_=adjT[jc * P:(jc + 1) * P, gsl],
                    )
                else:
                    nc.sync.dma_start(
                        out=at_t[:, :],
                        in_=adjT[jc * P:(jc + 1) * P, gsl],
                    )
                s2, s2b = s2p_sb[jc]
                xq = xp.tile([P, ISPAN], F16, tag="xq")
                if jc % relu_mod < relu_act_lt:
                    # ACT does add+relu (fused bias)
                    nc.scalar.activation(
                        xq[:, :], s1bc16[:, gsl], AF.Relu,
                        bias=s2[:, 0:1], scale=1.0,
                    )
                else:
                    # DVE: x+ = max(s1 + s2, 0) in one tensor_scalar
                    # (fp16 in/out -> 4x mode)
                    nc.vector.tensor_scalar(
                        xq[:, :], s1bc16[:, gsl], s2[:, 0:1], 0.0,
                        op0=ALU.add, op1=ALU.max,
                    )
                e = eep.tile([P, ISPAN], BF16, tag="e")
                nc.scalar.activation(
                    e[:, :], xq[:, :], AF.Exp,
                    bias=s2b[:, 0:1],
                    scale=1.0 if scale_fold else 1.0 - ALPHA,
                )
                pT = ptp.tile([P, ISPAN], BF16, tag="pT")
                nc.vector.tensor_tensor(
                    pT[:, :], e[:, :], at_t[:, :], op=ALU.mult
                )
                rh = h_sb[jc][:, 0:OUT + 1]
                for m in range(OCG):
                    nc.tensor.matmul(
                        accs[m][:, :],
                        pT[:, m * P:(m + 1) * P],
                        rh,
                        start=(jc == 0),
                        stop=(jc == JC - 1),
                    )

            def epilogue(g, accs):
                for m in range(OCG):
                    acc = accs[m]
                    r = ep.tile([P, 1], F32, tag="r")
                    nc.vector.reciprocal(r[:, :], acc[:, OUT:OUT + 1])
                    t = ep.tile([P, OUT], F32, tag="t")
                    nc.vector.tensor_scalar_mul(t[:, :], acc[:, 0:OUT], r[:, :])
                    # elu = relu(t) + (exp(-relu(-t)) - 1)
                    rn = ep.tile([P, OUT], F32, tag="rn")
                    en = ep.tile([P, OUT], F32, tag="en")
                    ps = ep.tile([P, OUT], F32, tag="ps")
                    if epi_dve:
                        nc.vector.tensor_scalar(
                            rn[:, :], t[:, :], -1.0, 0.0,
                            op0=ALU.mult, op1=ALU.max)
                        nc.scalar.activation(
                            en[:, :], rn[:, :], AF.Exp, scale=-1.0)
                        nc.vector.tensor_scalar_max(ps[:, :], t[:, :], 0.0)
                    else:
                        nc.scalar.activation(
                            rn[:, :], t[:, :], AF.Relu, scale=-1.0)
                        nc.scalar.activation(
                            en[:, :], rn[:, :], AF.Exp, scale=-1.0)
                        nc.scalar.activation(ps[:, :], t[:, :], AF.Relu)
                    res = ep.tile([P, OUT], F32, tag="res")
                    nc.vector.scalar_tensor_tensor(
                        res[:, :], ps[:, :], -1.0, en[:, :],
                        op0=ALU.add, op1=ALU.add,
                    )
                    row = (g * OCG + m) * P
                    nc.sync.dma_start(
                        out=out_own[row:row + P, :], in_=res[:, :]
                    )

            if interleave:
                # g=0: fused stage-1 + attention pass.  stage-1 leads by
                # lead2 jc-pairs so stage-2 consumes h/s2 produced several
                # iterations earlier instead of serializing on the fresh
                # cross-engine chain.
                gsl0 = slice(0, ISPAN)
                accs0 = [
                    pacc.tile([P, OUT + 1], F32, tag=f"acc{m}",
                              name=f"acc{m}")
                    for m in range(OCG)
                ]
                NJC2 = JC // 2
                for mi in range(NJC2 + lead2):
                    if mi < NJC2:
                        it2 = inp_p.tile([P, 2 * IN], BF16, tag="it",
                                         name="it")
                        nc.sync.dma_start(
                            out=it2[:, :],
                            in_=inpT_t[:, mi * 2 * IN:(mi + 1) * 2 * IN],
                        )
                        for a in range(2):
                            stage1_body(mi * 2 + a, it2, a)
                    if mi >= lead2:
                        jc2 = mi - lead2
                        for a in range(2):
                            stage2_body(0, gsl0, jc2 * 2 + a, accs0)
                epilogue(0, accs0)
                g_start = 1
            else:
                # plain stage 1 first
                for jc2 in range(JC // 2):
                    it2 = inp_p.tile([P, 2 * IN], BF16, tag="it", name="it")
                    nc.sync.dma_start(
                        out=it2[:, :],
                        in_=inpT_t[:, jc2 * 2 * IN:(jc2 + 1) * 2 * IN],
                    )
                    for a in range(2):
                        stage1_body(jc2 * 2 + a, it2, a)
                g_start = 0

            # remaining g groups: pure attention passes
            for g in range(g_start, G):
                gsl = slice(g * ISPAN, (g + 1) * ISPAN)
                accs = [
                    pacc.tile([P, OUT + 1], F32, tag=f"acc{m}", name=f"acc{m}")
                    for m in range(OCG)
                ]
                for jc in range(JC):
                    stage2_body(g, gsl, jc, accs)
                epilogue(g, accs)

    if split_waits:
        split_excess_waits(nc)
    return nc


# ---------------------------------------------------------------------------
# host wrapper
# ---------------------------------------------------------------------------
_CACHE = {}


def _get_nc(NJ, R, IN, OUT):
    key = (NJ, R, IN, OUT)
    if key not in _CACHE:
        _CACHE[key] = build_nc(NJ, R, IN, OUT)
    return _CACHE[key]


def _tile_inpT(inp_bf16, P=128):
    """[Nrows, IN] -> [128, (Nrows/P)*IN] where chunk (jc,kc) at cols
    jc*IN+kc*P..+P holds inpT[kc*P+p, jc*P+t] = inp[jc*P+t, kc*P+p]."""
    Nr, IN = inp_bf16.shape
    JC, KC = Nr // P, IN // P
    a = inp_bf16.reshape(JC, P, KC, P)        # [jc, t, kc, p]
    a = np.ascontiguousarray(a.transpose(3, 0, 2, 1))  # [p, jc, kc, t]
    return a.reshape(P, JC * IN)


def prep_in_maps(inp, adj, W, a1, a2, n_cores=N_CORES, adj_fp8=False):
    """Host-side layout prep: slicing + transposition + dtype casts only."""
    N, IN = inp.shape
    OUT = W.shape[1]
    R = N // n_cores
    bf16 = ml_dtypes.bfloat16
    inp_bf = inp.astype(bf16)
    inpT_t = _tile_inpT(inp_bf)
    W_bf = np.ascontiguousarray(W.astype(bf16))
    WT = np.ascontiguousarray(W.T)
    a12 = np.ascontiguousarray(np.concatenate([a1, a2], axis=1))
    adj_bf = adj.astype(ml_dtypes.float8_e4m3 if adj_fp8 else bf16)
    ident = np.eye(128, dtype=np.float32)
    in_maps = []
    for c in range(n_cores):
        sl = slice(c * R, (c + 1) * R)
        in_maps.append({
            "adjT": np.ascontiguousarray(adj_bf[sl, :].T),
            "inpT_t": inpT_t,
            "inpTo_t": _tile_inpT(inp_bf[sl, :]),
            "W": W_bf,
            "WT": WT,
            "a12": a12,
            "ident": ident,
        })
    return in_maps, R, IN, OUT


def kernel(inp, adj, W, a1, a2):
    inp = np.asarray(inp, dtype=np.float32)
    adj = np.asarray(adj, dtype=np.int32)
    W = np.asarray(W, dtype=np.float32)
    a1 = np.asarray(a1, dtype=np.float32)
    a2 = np.asarray(a2, dtype=np.float32)
    N = inp.shape[0]
    in_maps, R, IN, OUT = prep_in_maps(inp, adj, W, a1, a2)
    nc = _get_nc(N, R, IN, OUT)
    res = run_bass_kernel_spmd(nc, in_maps, list(range(N_CORES)))
    return np.concatenate(
        [res.results[c]["out"] for c in range(N_CORES)], axis=0
    )
